# revision 17
# baseline (speedup 1.0000x reference)
"""Trainium2 Bass kernel for nn_LogMarginalLikelihood (GP log-marginal-likelihood
via batched CG + stochastic Lanczos quadrature).

Self-contained: hardcodes shapes N=8192, 101 RHS columns (y + 100 probes) on
128 SBUF lanes (lanes 101..127 unused), PIT=12 Krylov iterations (validated:
exact-arithmetic SLQ with p=12 matches p=30 to ~5e-14; fp16 device arithmetic
lands ~1e-5), 8-way column sharding of the symmetric K.

Algorithm: Ghysels-Vanroose pipelined CG, one tiny dot-product AllGather per
iteration (hidden under the matvec) plus the w-transport AllGather, split in
two halves that are overlapped with the matvec via split-phase emission:

 - matvec k runs phase t=0 (output shard rows 0:512, all 64 contraction
   blocks) then phase t=1, each into its own PSUM bank.
 - Shard rows 0:512 are the core's local natural blocks 0..3 = AG-a payload.
   The PE instruction stream is ordered [ph0(k) | transposes-a(k+1) | ph1(k)
   | transposes-b(k+1) | ph0(k+1) | ...], so AG-a(k+1) is triggered while
   ph1(k) runs and its CC time + pnat loads hide under PE work.
 - Contraction blocks are host-permuted mod 8 so blocks 0..31 are AG-a's
   content: ph0(k+1) starts as soon as AG-a(k+1) lands; AG-b(k+1) arrives
   before block-slot 32 of ph0(k+1) is reached.
 - PSUM WAR is released per bank (bank-t consumers run during the other
   phase), keeping the PE nearly gap-free.

Recurrences (per column):
    beta_k  = g_k/g_{k-1}              g_k = (r_k, r_k)
    alpha_k = g_k/(d_k - beta_k*g_k/alpha_{k-1})   d_k = (w_k, r_k), w = A r
    z = m + beta*z   (m = A w)         s = w + beta*s    (s = A p)
    r' = r - alpha*s                   w' = w - alpha*z
Critical-path form: w' = (-alpha)*m + t1 with t1 = w - (alpha*beta)*z
computed during the matvec; z' = m + beta*z recomputed off the critical path.

Device outputs the g/d histories; the host rebuilds alphas/betas
(beta_k = g_{k+1}/g_k in the reference convention), y^T K^-1 y = sum alpha_k g_k,
and the SLQ logdet via batched eigh of the 12x12 Lanczos T matrices.

Transport scaling: w_k is cast to fp16 as w_k/sw_k with sw_k predicted
device-side from the gamma history (sw_k = sqrt(g_{k-1}^2/g_{k-2})*F), so no
extra collective is needed; mis-prediction only shifts the fp16 exponent.

PE packing: column groups of 64 (PE cols 0..63) and 37 (PE cols 64..100) via
tile_position col-tiling so LDWEIGHTS of one group hides under the other
group's matmul.
"""

import numpy as np

N = 8192
TREAL = 101        # 1 solve column (y) + 100 probes
TL = 128           # SBUF lane count (tiles padded; lanes TREAL.. unused)
G1 = 64            # column group 0 width (PE cols 0..63)
G2 = TREAL - G1    # column group 1 width = 37 (PE cols 64..100)
PIT = 12           # Krylov iterations (coefficient count fed to SLQ)
NCORES = 8
SH = N // NCORES   # 1024 K-columns per core
SHH = SH // 2      # 512 shard rows per transport half
NB = N // 128      # 64 contraction blocks
NBH = NB // 2      # 32 blocks per AllGather half
F = 16.0           # |A r| / |r| headroom factor for fp16 transport scaling
EPS = 1e-35

# contraction-block permutation: first all blocks = 0..3 (mod 8) in rank-major
# order (delivered by AG-a), then blocks = 4..7 (mod 8) (delivered by AG-b)
PERM = ([8 * c + j for c in range(NCORES) for j in range(4)]
        + [8 * c + 4 + j for c in range(NCORES) for j in range(4)])

_cached = {}


def _build():
    import concourse.bacc as bacc
    import concourse.tile as tile
    from concourse import mybir

    fp32 = mybir.dt.float32
    fp16 = mybir.dt.float16
    Alu = mybir.AluOpType
    Act = mybir.ActivationFunctionType
    X = mybir.AxisListType.X

    nc = bacc.Bacc(None, target_bir_lowering=False, num_devices=NCORES)

    k_shard = nc.dram_tensor("k_shard", [N, SH], fp16, kind="ExternalInput")
    ident_in = nc.dram_tensor("ident", [128, 128], fp16, kind="ExternalInput")
    bt_in = nc.dram_tensor("bt", [TREAL, SH], fp32, kind="ExternalInput")
    bnat_in = nc.dram_tensor("bnat", [N, TREAL], fp16, kind="ExternalInput")
    s0_in = nc.dram_tensor("s0v", [TL, 1], fp32, kind="ExternalInput")
    swf_in = nc.dram_tensor("swf", [TL, 1], fp32, kind="ExternalInput")
    swif_in = nc.dram_tensor("swif", [TL, 1], fp32, kind="ExternalInput")
    gh_out = nc.dram_tensor("gh", [TL, PIT + 1], fp32, kind="ExternalOutput")
    dh_out = nc.dram_tensor("dh", [TL, PIT + 1], fp32, kind="ExternalOutput")

    rg = [list(range(NCORES))]
    A_ = slice(0, TREAL)          # active lanes
    A1 = slice(0, G1)             # group-0 lanes
    A2 = slice(G1, TREAL)         # group-1 lanes

    with tile.TileContext(nc) as tc:
        with (
            tc.tile_pool(name="kpool", bufs=1) as kpool,
            tc.tile_pool(name="persist", bufs=1) as persist,
            tc.tile_pool(name="state", bufs=2) as state,
            tc.tile_pool(name="work", bufs=1) as work,
            tc.tile_pool(name="small", bufs=1) as small,
            tc.tile_pool(name="vt0_ps", bufs=1, space="PSUM") as vt0_pool,
            tc.tile_pool(name="vt1_ps", bufs=1, space="PSUM") as vt1_pool,
            tc.tile_pool(name="tr_ps", bufs=2, space="PSUM") as tr_pool,
            tc.tile_pool(name="dram", bufs=2, space="DRAM") as dram,
        ):
            # ---- one-time loads (small inputs first so init matvec can
            # chase the ksb chunks) ----
            ident = persist.tile([128, 128], fp16)
            nc.sync.dma_start(ident[:], ident_in[:])
            s0v = persist.tile([TL, 1], fp32, name="s0v")
            swf = persist.tile([TL, 1], fp32, name="swf")
            swif = persist.tile([TL, 1], fp32, name="swif")
            nc.sync.dma_start(s0v[:], s0_in[:])
            nc.sync.dma_start(swf[:], swf_in[:])
            nc.sync.dma_start(swif[:], swif_in[:])
            gh = persist.tile([TL, PIT + 1], fp32, name="gh_sb")
            dh = persist.tile([TL, PIT + 1], fp32, name="dh_sb")

            r = state.tile([TL, SH], fp32, name="R_0", tag="R")
            nc.sync.dma_start(r[A_, :], bt_in[:])
            bv = bnat_in.rearrange("(b p) t -> p b t", p=128)
            p0a = state.tile([128, NBH, G1], fp16, name="p0a_0", tag="P0a",
                             bufs=1)
            p0b = state.tile([128, NBH, G1], fp16, name="p0b_0", tag="P0b",
                             bufs=1)
            p1a = state.tile([128, NBH, G2], fp16, name="p1a_0", tag="P1a",
                             bufs=1)
            p1b = state.tile([128, NBH, G2], fp16, name="p1b_0", tag="P1b",
                             bufs=1)
            for h in range(2):
                cs = slice(16 * h, 16 * h + 16)
                cb = slice(32 + 16 * h, 48 + 16 * h)
                nc.sync.dma_start(p0a[:, cs, :], bv[:, cs, 0:G1])
                nc.sync.dma_start(p1a[:, cs, :], bv[:, cs, G1:TREAL])
                nc.sync.dma_start(p0b[:, cs, :], bv[:, cb, 0:G1])
                nc.sync.dma_start(p1b[:, cs, :], bv[:, cb, G1:TREAL])

            ksb = kpool.tile([128, NB, SH], fp16)
            kv = k_shard.rearrange("(b p) i -> p b i", p=128)
            for b in range(8):
                nc.sync.dma_start(ksb[:, 8 * b:8 * b + 8, :],
                                  kv[:, 8 * b:8 * b + 8, :])

            def emit_matvec_phase(vt0, vt1, t):
                for b in range(NB):
                    ph0 = p0a if b < NBH else p0b
                    ph1 = p1a if b < NBH else p1b
                    bb = b % NBH
                    nc.tensor.matmul(
                        vt0[0:G1, t, :], ph0[:, bb, :],
                        ksb[:, b, 512 * t:512 * t + 512],
                        start=(b == 0), stop=(b == NB - 1),
                        tile_position=(0, 0))
                    nc.tensor.matmul(
                        vt1[G1:TREAL, t, :], ph1[:, bb, :],
                        ksb[:, b, 512 * t:512 * t + 512],
                        start=(b == 0), stop=(b == NB - 1),
                        tile_position=(0, 64))

            def new_vt(tag_k):
                vt0 = vt0_pool.tile([128, 2, 512], fp32, name=f"vt0_{tag_k}",
                                    tag="vt0")
                vt1 = vt1_pool.tile([128, 2, 512], fp32, name=f"vt1_{tag_k}",
                                    tag="vt1")
                return vt0, vt1

            def emit_transport_half(wt16, half, tag_k):
                # half 0: natural local blocks 0..3 (shard rows 0:512, AG-a)
                # half 1: natural local blocks 4..7 (shard rows 512:1024, AG-b)
                pref = "ab"[half]
                pn_sh = work.tile([128, 4, TREAL], fp16, tag=f"pn_{pref}")
                for j in range(4):
                    jj = 4 * half + j
                    trp = tr_pool.tile([128, TREAL], fp16, tag="trp")
                    nc.tensor.transpose(
                        trp[:], wt16[A_, 128 * jj:128 * jj + 128],
                        ident[A_, A_])
                    nc.vector.tensor_copy(pn_sh[:, j, :], trp[:])
                agi = dram.tile([SHH, TREAL], fp16, tag=f"ag_{pref}i")
                ago = dram.tile([N // 2, TREAL], fp16, tag=f"ag_{pref}o",
                                addr_space="Shared")
                nc.sync.dma_start(
                    agi.rearrange("(j p) t -> p j t", p=128), pn_sh[:])
                nc.gpsimd.collective_compute(
                    "AllGather", Alu.bypass, replica_groups=rg,
                    ins=[agi.opt()], outs=[ago.opt()])
                return ago

            def emit_loads(ago, half, tag_k):
                pref = "ab"[half]
                n0 = state.tile([128, NBH, G1], fp16, name=f"p0{pref}_{tag_k}",
                                tag=f"P0{pref}", bufs=1)
                n1 = state.tile([128, NBH, G2], fp16, name=f"p1{pref}_{tag_k}",
                                tag=f"P1{pref}", bufs=1)
                agv = ago.rearrange("(b p) t -> p b t", p=128)
                for h in range(2):
                    cs = slice(16 * h, 16 * h + 16)
                    nc.sync.dma_start(n0[:, cs, :], agv[:, cs, 0:G1])
                    nc.sync.dma_start(n1[:, cs, :], agv[:, cs, G1:TREAL])
                return n0, n1

            def emit_dots(r_t, w_t, tag_k):
                scr = work.tile([TL, SH], fp32, tag="scr")
                gd2 = small.tile([TL, 2], fp32, tag="gd2")
                nc.vector.tensor_tensor(scr[A_, :], r_t[A_, :], r_t[A_, :],
                                        Alu.mult)
                nc.vector.tensor_reduce(gd2[A_, 0:1], scr[A_, :], X, Alu.add)
                nc.vector.tensor_tensor(scr[A_, :], w_t[A_, :], r_t[A_, :],
                                        Alu.mult)
                nc.vector.tensor_reduce(gd2[A_, 1:2], scr[A_, :], X, Alu.add)
                agsi = dram.tile([TL, 2], fp32, tag="agsi")
                agso = dram.tile([NCORES, TL, 2], fp32, tag="agso",
                                 addr_space="Shared")
                nc.sync.dma_start(agsi[A_, :], gd2[A_, :])
                nc.gpsimd.collective_compute(
                    "AllGather", Alu.bypass, replica_groups=rg,
                    ins=[agsi.opt()], outs=[agso.opt()])
                return agso

            def consume_small_ag(agso, k):
                gd_all = small.tile([TL, 2, NCORES], fp32, tag="gd_all")
                nc.sync.dma_start(gd_all[A_, :, :],
                                  agso.rearrange("r p c -> p c r")[A_, :, :])
                nc.vector.tensor_reduce(gh[A_, k:k + 1], gd_all[A_, 0:1, :],
                                        X, Alu.add)
                nc.vector.tensor_reduce(dh[A_, k:k + 1], gd_all[A_, 1:2, :],
                                        X, Alu.add)

            # ---- init: w0 = A r0 (input pre-scaled by 1/s0) ----
            vt0, vt1 = new_vt("init")
            emit_matvec_phase(vt0, vt1, 0)
            w = state.tile([TL, SH], fp32, name="W_0", tag="W")
            wt16 = work.tile([TL, SH], fp16, tag="wt16")
            # half 0 of w0 ready after phase 0; transport-a overlaps phase 1
            nc.vector.tensor_scalar_mul(w[A1, 0:SHH], vt0[A1, 0, :],
                                        s0v[A1, :])
            nc.vector.tensor_scalar_mul(w[A2, 0:SHH], vt1[A2, 0, :],
                                        s0v[A2, :])
            nc.vector.tensor_scalar_mul(wt16[A_, 0:SHH], w[A_, 0:SHH],
                                        swif[A_, :])
            ago_a = emit_transport_half(wt16, 0, "t0")
            emit_matvec_phase(vt0, vt1, 1)
            nc.vector.tensor_scalar_mul(w[A1, SHH:SH], vt0[A1, 1, :],
                                        s0v[A1, :])
            nc.vector.tensor_scalar_mul(w[A2, SHH:SH], vt1[A2, 1, :],
                                        s0v[A2, :])
            nc.vector.tensor_scalar_mul(wt16[A_, SHH:SH], w[A_, SHH:SH],
                                        swif[A_, :])
            ago_b = emit_transport_half(wt16, 1, "t0")
            p0a, p1a = emit_loads(ago_a, 0, "t0")
            p0b, p1b = emit_loads(ago_b, 1, "t0")
            agso = emit_dots(r, w, "init")
            vt0_n, vt1_n = new_vt("mv0")
            emit_matvec_phase(vt0_n, vt1_n, 0)
            vt0_p, vt1_p = vt0, vt1          # previous (fully consumed)
            vt0, vt1 = vt0_n, vt1_n

            z = None
            s = None
            alpha_prev = None
            swc = swf
            swic = swif

            for k in range(PIT):
                do_zw = k <= PIT - 2        # need w update (have m_k)
                do_next = k <= PIT - 3      # need transport + next matvec

                # -- consume small AG k (overlaps ph0(k)) --
                consume_small_ag(agso, k)

                # -- alpha/beta tiny chain (overlaps ph0(k)) --
                g_k = gh[:, k:k + 1]
                d_k = dh[:, k:k + 1]
                beta = small.tile([TL, 1], fp32, tag="beta")
                alpha = small.tile([TL, 1], fp32, name=f"al_{k}", tag="alpha",
                                   bufs=2)
                t0 = small.tile([TL, 1], fp32, tag="t0")
                t0i = small.tile([TL, 1], fp32, tag="t0i")
                if k == 0:
                    nc.vector.tensor_scalar_mul(beta[:], g_k, 0.0)
                    nc.vector.tensor_scalar_add(t0[:], d_k, EPS)
                    nc.vector.reciprocal(t0i[:], t0[:])
                    nc.vector.tensor_tensor(alpha[:], g_k, t0i[:], Alu.mult)
                else:
                    g_km1 = gh[:, k - 1:k]
                    nc.vector.tensor_scalar_add(t0[:], g_km1, EPS)
                    nc.vector.reciprocal(t0i[:], t0[:])
                    nc.vector.tensor_tensor(beta[:], g_k, t0i[:], Alu.mult)
                    ap1 = small.tile([TL, 1], fp32, tag="ap1")
                    ap1i = small.tile([TL, 1], fp32, tag="ap1i")
                    nc.vector.tensor_scalar_add(ap1[:], alpha_prev[:], EPS)
                    nc.vector.reciprocal(ap1i[:], ap1[:])
                    u = small.tile([TL, 1], fp32, tag="u")
                    nc.vector.tensor_tensor(u[:], g_k, ap1i[:], Alu.mult)
                    q1 = small.tile([TL, 1], fp32, tag="q1")
                    nc.vector.tensor_tensor(q1[:], beta[:], u[:], Alu.mult)
                    q2 = small.tile([TL, 1], fp32, tag="q2")
                    nc.vector.tensor_tensor(q2[:], d_k, q1[:], Alu.subtract)
                    nc.vector.tensor_scalar_add(q2[:], q2[:], EPS)
                    q2i = small.tile([TL, 1], fp32, tag="q2i")
                    nc.vector.reciprocal(q2i[:], q2[:])
                    nc.vector.tensor_tensor(alpha[:], g_k, q2i[:], Alu.mult)
                nalpha = small.tile([TL, 1], fp32, tag="nalpha")
                nc.vector.tensor_scalar_mul(nalpha[:], alpha[:], -1.0)
                nab = small.tile([TL, 1], fp32, tag="nab")
                nc.vector.tensor_tensor(nab[:], nalpha[:], beta[:], Alu.mult)
                nasw = small.tile([TL, 1], fp32, tag="nasw")
                nc.vector.tensor_tensor(nasw[:], nalpha[:], swc[:], Alu.mult)

                # -- transport scale for w_{k+1} (k>=1: ghat = g_k^2/g_{k-1}) --
                if k >= 1 and do_next:
                    ghat = small.tile([TL, 1], fp32, tag="ghat")
                    nc.vector.tensor_tensor(ghat[:], beta[:], g_k, Alu.mult)
                    sq = small.tile([TL, 1], fp32, tag="sq")
                    nc.scalar.activation(sq[:], ghat[:], Act.Sqrt)
                    nswc = small.tile([TL, 1], fp32, name=f"swc_{k}",
                                      tag="swc", bufs=2)
                    nc.vector.tensor_scalar_mul(nswc[:], sq[:], F)
                    nswic = small.tile([TL, 1], fp32, name=f"swic_{k}",
                                       tag="swic", bufs=2)
                    ep = small.tile([TL, 1], fp32, tag="ep")
                    nc.vector.tensor_scalar_add(ep[:], nswc[:], EPS)
                    nc.vector.reciprocal(nswic[:], ep[:])
                else:
                    nswc, nswic = swc, swic

                # -- t1 = w - (alpha*beta) z, zb = beta*z: during ph0(k) --
                if do_zw and k >= 1:
                    t1 = state.tile([TL, SH], fp32, name=f"T1_{k}", tag="T1",
                                    bufs=1)
                    nc.vector.scalar_tensor_tensor(
                        t1[A_, :], z[A_, :], nab[:TREAL, :], w[A_, :],
                        Alu.mult, Alu.add)
                    zb = state.tile([TL, SH], fp32, name=f"Zb_{k}", tag="Zb",
                                    bufs=1)
                    nc.vector.tensor_scalar_mul(zb[A_, :], z[A_, :],
                                                beta[:TREAL, :])
                else:
                    t1 = w
                    zb = None

                if do_zw:
                    wn = state.tile([TL, SH], fp32, name=f"W_{k + 1}", tag="W")
                    wt16 = work.tile([TL, SH], fp16, tag="wt16")
                    zn = state.tile([TL, SH], fp32, name=f"Z_{k}", tag="Z")
                else:
                    wn = w
                    zn = None

                agos = [None, None]
                for t in range(2):
                    cs = slice(SHH * t, SHH * t + SHH)
                    if do_zw:
                        # wn half t: needs matvec-k phase t only
                        nc.vector.scalar_tensor_tensor(
                            wn[A1, cs], vt0[A1, t, :], nasw[A1, :],
                            t1[A1, cs], Alu.mult, Alu.add)
                        nc.vector.scalar_tensor_tensor(
                            wn[A2, cs], vt1[A2, t, :], nasw[A2, :],
                            t1[A2, cs], Alu.mult, Alu.add)
                    if do_next:
                        nc.vector.tensor_scalar_mul(
                            wt16[A_, cs], wn[A_, cs], nswic[A_, :])
                        agos[t] = emit_transport_half(wt16, t, f"t{k + 1}")
                    if do_zw:
                        # z half t: frees PSUM bank t for the next matvec
                        if k == 0:
                            nc.vector.tensor_scalar_mul(
                                zn[A1, cs], vt0[A1, t, :], swc[A1, :])
                            nc.vector.tensor_scalar_mul(
                                zn[A2, cs], vt1[A2, t, :], swc[A2, :])
                        else:
                            nc.vector.scalar_tensor_tensor(
                                zn[A1, cs], vt0[A1, t, :], swc[A1, :],
                                zb[A1, cs], Alu.mult, Alu.add)
                            nc.vector.scalar_tensor_tensor(
                                zn[A2, cs], vt1[A2, t, :], swc[A2, :],
                                zb[A2, cs], Alu.mult, Alu.add)
                    if t == 0 and do_zw:
                        # emit ph1(k) after transport-a so the PE runs
                        # [ph0(k) | transp-a | ph1(k) | transp-b | ph0(k+1)]
                        emit_matvec_phase(vt0, vt1, 1)
                if do_zw:
                    z = zn

                if do_next:
                    p0a, p1a = emit_loads(agos[0], 0, f"t{k + 1}")
                    p0b, p1b = emit_loads(agos[1], 1, f"t{k + 1}")

                # -- s/r updates + dots (during AG / early next matvec) --
                sn = state.tile([TL, SH], fp32, name=f"S_{k}", tag="S")
                if k == 0:
                    nc.vector.tensor_copy(sn[A_, :], w[A_, :])
                else:
                    nc.vector.scalar_tensor_tensor(
                        sn[A_, :], s[A_, :], beta[:TREAL, :], w[A_, :],
                        Alu.mult, Alu.add)
                rn = state.tile([TL, SH], fp32, name=f"R_{k + 1}", tag="R")
                nc.vector.scalar_tensor_tensor(
                    rn[A_, :], sn[A_, :], nalpha[:TREAL, :], r[A_, :],
                    Alu.mult, Alu.add)
                s = sn
                r = rn
                w = wn

                agso = emit_dots(r, w, f"d{k + 1}")

                if do_next:
                    vt0_n, vt1_n = new_vt(f"mv{k + 1}")
                    emit_matvec_phase(vt0_n, vt1_n, 0)
                    vt0, vt1 = vt0_n, vt1_n

                alpha_prev = alpha
                swc, swic = nswc, nswic

            # final gamma_{PIT}
            consume_small_ag(agso, PIT)
            nc.sync.dma_start(gh_out[:], gh[:])
            nc.sync.dma_start(dh_out[:], dh[:])

    nc.compile()
    return nc


def _get_nc():
    if "nc" not in _cached:
        _cached["nc"] = _build()
    return _cached["nc"]


def kernel(Knn_noise: np.ndarray, y: np.ndarray, Z: np.ndarray) -> np.ndarray:
    from concourse.bass_utils import run_bass_kernel_spmd

    K = np.ascontiguousarray(Knn_noise, dtype=np.float32)
    B = np.concatenate([y.astype(np.float32), Z.astype(np.float32)], axis=1)
    g0 = np.sum(B.astype(np.float64) * B.astype(np.float64), axis=0)
    s0 = np.sqrt(g0)
    bnat = (B / s0[None, :]).astype(np.float16)
    K16 = K.astype(np.float16)
    # permute contraction (row) blocks so AG-half-a blocks come first
    K16p = np.ascontiguousarray(
        K16.reshape(NB, 128, N)[PERM].reshape(N, N))
    bnatp = np.ascontiguousarray(
        bnat.reshape(NB, 128, TREAL)[PERM].reshape(N, TREAL))
    BT = np.ascontiguousarray(B.T.astype(np.float32))
    ident = np.eye(128, dtype=np.float16)

    def padlane(v):
        out = np.ones((TL, 1), np.float32)
        out[:TREAL, 0] = v
        return out

    s0v = padlane(s0)
    swf = padlane(s0 * F)
    swif = padlane(1.0 / (s0 * F))

    in_maps = []
    for c in range(NCORES):
        m = {"k_shard": np.ascontiguousarray(K16p[:, SH * c:SH * (c + 1)]),
             "ident": ident,
             "bt": np.ascontiguousarray(BT[:, SH * c:SH * (c + 1)]),
             "bnat": bnatp,
             "s0v": s0v, "swf": swf, "swif": swif}
        in_maps.append(m)

    nc = _get_nc()
    _cached["last_in_maps"] = in_maps
    res = run_bass_kernel_spmd(nc, in_maps, core_ids=list(range(NCORES)))
    out0 = res.results[0]
    gams = out0["gh"].astype(np.float64)[:TREAL, :PIT + 1].T  # [PIT+1, TREAL]
    dels = out0["dh"].astype(np.float64)[:TREAL, :PIT + 1].T

    # host-side coefficient extraction (pipelined-CG recurrences)
    alphas = np.zeros((PIT, TREAL))
    al_p = None
    for k in range(PIT):
        if k == 0:
            al = gams[0] / dels[0]
        else:
            be = gams[k] / gams[k - 1]
            al = gams[k] / (dels[k] - be * gams[k] / al_p)
        alphas[k] = al
        al_p = al
    betas = gams[1:PIT + 1] / gams[:PIT]   # reference convention

    yKiy = float(np.sum(alphas[:, 0] * gams[:PIT, 0]))

    a = alphas[:, 1:TREAL]
    b = betas[:, 1:TREAL]
    inv_a = 1.0 / a
    diag = inv_a.copy()
    diag[1:] += b[:-1] / a[:-1]
    off = np.sqrt(np.maximum(b[:-1], 0.0)) / a[:-1]
    Ts_m = np.zeros((TREAL - 1, PIT, PIT))
    idx = np.arange(PIT)
    Ts_m[:, idx, idx] = diag.T
    Ts_m[:, idx[:-1], idx[1:]] = off.T
    Ts_m[:, idx[1:], idx[:-1]] = off.T
    lam, V = np.linalg.eigh(Ts_m)
    lam = np.maximum(lam, 1e-12)
    quad = np.sum(V[:, 0, :] ** 2 * np.log(lam), axis=1)
    log_det = N * float(np.mean(quad))

    out = -0.5 * yKiy - 0.5 * log_det - N * 0.5 * np.log(2.0 * np.pi)
    return np.array([[out]], dtype=np.float32)
